# revision 33
# baseline (speedup 1.0000x reference)
"""Trainium2 Bass kernel for a 12-head attention layer (ViT-style, N=577).

Reference computation (fp32):
    qkv = x @ w_qkv            [B,N,3E]
    q,k,v per head (H=12, Dh=64)
    att = softmax(q k^T / sqrt(Dh))
    out = (att v) concat heads @ w_proj + b_proj

Sharding: data-parallel over batch across 8 NeuronCores (4 batch items per
core), weights replicated, no collectives; outputs concatenated on the host.

Precision: x and the weights are converted to fp16 on the HOST (the PE
consumed fp16 operands anyway, so numerics are unchanged); accumulation is
fp32 in PSUM. The v columns of w_qkv are pre-scaled by 1/256 on the host,
with a matching 1/256 ones column in-kernel, so the unnormalized attention
sums and softmax denominators fit fp16 range -- the scale cancels exactly
in the normalize divide, and the whole post-PSUM normalization chain runs
in fp16 (2x/4x DVE modes, half the GpSimd broadcast bytes).

Design notes -- the Tensor engine's clock ramps (1.2GHz -> 2.4GHz after
~3us of continuous execution) and drops back on idle, so the kernel is
structured to keep PE busy with independent work at every stall point:

  1. x_b loaded natural one batch ahead (SP queue, fp16 so DMA bytes are
     halved; weights stream straight into their compute tiles on both
     HWDGE queues -- no staging or on-core conversion), transposed on PE
     (1 cycle/row fp16) -> xT [768,577]. The next batch's per-tile
     transpose is spread across head-pair iterations 0..4.
  2. qT,kT head-pair-wise: lhsT=w_qkv cols, rhs=xT, psum split 512+65
     (bank limit), one fp16 DVE copy-out. v in natural token layout
     [tok, 12, 64+1] fp16 with a 1/256 column per head.
  3. per head: scoresT[j,i] = kT^T qT (K=64). exp fused with the 1/8
     attention scale on ScalarE (PSUM->SBUF, fp16; scores are O(+-6) so no
     max-subtraction).
  4. att@v of a PENDING head is emitted immediately after a pair's q/k
     matmuls: its ~1.2us of PE work covers the DVE psum->SBUF pair copies
     so the first score ldweights never waits on them; a second pending
     head goes between the two subs (pure filler). att@v in outT form:
     lhsT = v_ext [j,65] fp16, rhs = attT [j,i] -> psum [65, i]; row 64 is
     the softmax denominator (1/256 column). Raw block + denominator are
     copied off psum in ONE fp16 DVE copy; normalization is deferred one
     head: per-head partition_broadcast of the denom row on the idle
     GpSimd engine, fp16 reciprocal + fp16 multiply on DVE into aoT.
  5. the next batch's v matmuls are emitted between the attv tail and the
     normalize+proj block, so PE chews on v while the normalization chain
     (GpSimd broadcast -> DVE recip+mul) completes.
  6. proj: lhsT = aoT chunks, rhs = w_proj fp16; bias added via DVE on the
     PSUM->SBUF copy; DMA out in natural layout (fp32).

Build notes (hard-won):
  - Must build with Bacc and call nc.compile(): it redistributes semaphore
    waits (HW allows 1 wait per instruction) onto ldweights/event-semaphore
    carriers. Plain Bass + TileContext emits multi-wait instructions that
    walrus rejects ("Too many sync wait commands").
  - A dummy transpose up front makes PE observe the gpsimd semaphore once
    so the first real transpose doesn't need two waits on its LW slot.
"""

import numpy as np

import concourse.bass as bass
import concourse.bacc as bacc
import concourse.tile as tile
from concourse import mybir
from concourse.bass_utils import run_bass_kernel_spmd
from concourse.masks import make_identity

# Problem shape (hardcoded per contract)
B, N, E = 32, 577, 768
H, D = 12, 64
F3 = 3 * E
NCORES = 8
BL = B // NCORES  # batch per core
SCALE = float(D) ** -0.5

FP = mybir.dt.float32
HF = mybir.dt.float16

# token chunking: 577 = 4*128 + 65
TCH = [(i * 128, min(128, N - i * 128)) for i in range((N + 127) // 128)]
KE = E // 128  # 6 contraction chunks over embed dim

# psum free-dim splits: a matmul output must stay inside one 2KB psum bank
# (512 fp32), so the 577-wide dims split 512 + 65 (fp16 operands run 1
# cycle/row at any width).
NCH = [(0, 512), (512, 65)]
ECH = [(0, 512), (512, 256)]  # 768 output features


def _emit(tc, x, w_qkv, w_proj, b_proj, y, ctx):
    nc = tc.nc

    # ---- pools ----
    wq_pool = ctx.enter_context(tc.tile_pool(name="wq", bufs=KE))
    wp_pool = ctx.enter_context(tc.tile_pool(name="wp", bufs=KE))
    const_pool = ctx.enter_context(tc.tile_pool(name="const", bufs=1))
    x_pool = ctx.enter_context(tc.tile_pool(name="xin", bufs=8))
    xt_pool = ctx.enter_context(tc.tile_pool(name="xt", bufs=2 * KE))
    qk_pool = ctx.enter_context(tc.tile_pool(name="qk", bufs=6))
    v_pool = ctx.enter_context(tc.tile_pool(name="v", bufs=2 * len(TCH)))
    att_pool = ctx.enter_context(tc.tile_pool(name="att", bufs=3 * len(TCH)))
    araw_pool = ctx.enter_context(tc.tile_pool(name="araw", bufs=7))
    aot_pool = ctx.enter_context(tc.tile_pool(name="aot", bufs=KE + 1))
    y_pool = ctx.enter_context(tc.tile_pool(name="y", bufs=3))
    rbc_pool = ctx.enter_context(tc.tile_pool(name="rbc", bufs=7))

    # PSUM: 8 banks of [128, 2KB]. tag p2: matmul accumulators ([128,768] =
    # 2 banks x 3 bufs); tag pstx: transpose staging (fp16, 2 bufs).
    ps1 = ctx.enter_context(tc.tile_pool(name="ps1", bufs=1, space="PSUM"))

    # ---- constants ----
    ident = const_pool.tile([128, 128], HF, name="ident", tag="ident")
    make_identity(nc, ident)

    # Dummy transpose so PE observes the gpsimd (Pool) semaphore once, up
    # front: walrus's matmul load-weights slot fits only ONE sync wait, and
    # without this the first real transpose would need Pool + DMA waits.
    warm = ps1.tile([128, 512], HF, name="warm", tag="pstx", bufs=2)
    nc.tensor.transpose(warm[:128, :128], ident[:, :], ident[:, :])

    def load_x(b):
        # issue the DMAs for batch b's x tiles (SP queue); x is fp16 host-side
        xin = []
        for ti, (ts_, tw) in enumerate(TCH):
            t = x_pool.tile([128, E], HF, name="xin", tag="xin")
            nc.sync.dma_start(t[:tw, :], x[b, ts_ : ts_ + tw, :])
            xin.append(t)
        return xin

    def alloc_xT():
        return [xt_pool.tile([128, N], HF, name=f"xT{kc}", tag="xT") for kc in range(KE)]

    def transpose_x_tile(xin, xT, ti):
        # PE-transpose one (already fp16) x tile into xT columns
        ts_, tw = TCH[ti]
        for ec in range(KE):
            pst = ps1.tile([128, 512], HF, name="pst", tag="pstx", bufs=2)
            nc.tensor.transpose(
                pst[:128, :tw], xin[ti][:tw, ec * 128 : (ec + 1) * 128], ident[:tw, :tw]
            )
            nc.vector.tensor_copy(xT[ec][:, ts_ : ts_ + tw], pst[:128, :tw])

    def emit_v(xT):
        # v in natural layout [tok, 12, 64+1] fp16 (ones column per head)
        v_t = []
        for ti, (ts_, tw) in enumerate(TCH):
            psv0 = ps1.tile([128, E], FP, name="psv0", tag="p2", bufs=3)
            psv = [psv0[:, fs : fs + fw] for (fs, fw) in ECH]
            for kc in range(KE):
                for ci, (fs, fw) in enumerate(ECH):
                    nc.tensor.matmul(
                        psv[ci][:tw, :fw],
                        xT[kc][:, ts_ : ts_ + tw],
                        wv_t[kc][:, fs : fs + fw],
                        start=(kc == 0),
                        stop=(kc == KE - 1),
                    )
            # v_ext column layout [ones, 63 zeros, v(64)]: the ones column
            # FIRST puts the softmax denominator in psum row 0 of att@v
            # (partition_broadcast can only read partition 0) and the zero
            # pad aligns the raw block to partition 64 (SBUF accesses must
            # start at an aligned base partition)
            vt = v_pool.tile([128, H, 2 * D], HF, name="v", tag="v")
            # ONE drain copy across both psum banks: psum frees in a single
            # retire and the ACT instruction count halves
            nc.scalar.copy(
                vt[:tw, :, D : 2 * D],
                psv0[:tw, :E].rearrange("p (h d) -> p h d", d=D),
            )
            # v columns are pre-scaled by 1/256 on the host; the matching
            # 1/256 ones column keeps denominator/raw-output in fp16 range
            # (the scale cancels exactly in the normalize divide)
            nc.vector.memset(vt[:tw, :, 0:1], 1.0 / 256.0)
            nc.gpsimd.memset(vt[:tw, :, 1:D], 0.0)
            v_t.append(vt)
        return v_t

    # x for batch 0 first: nothing can start before it lands.
    xin_next = load_x(0)

    # weights are fp16 host-side (v columns pre-scaled by 1/256): DMA lands
    # straight in the compute tiles, no staging or on-core conversion.
    # Streamed on BOTH hwdge queues (alternating); q/k columns ship before v
    # columns so batch 0's first head-pair starts as early as possible.
    QKW = 2 * E
    wqk_t = []
    for kc in range(KE):
        t = wq_pool.tile([128, QKW], HF, name=f"wqk{kc}", tag="wqk")
        eng = nc.sync if kc % 2 else nc.scalar
        eng.dma_start(t[:, :], w_qkv[kc * 128 : (kc + 1) * 128, 0:QKW])
        wqk_t.append(t)
    wv_t = []
    for kc in range(KE):
        t = wq_pool.tile([128, E], HF, name=f"wv{kc}", tag="wv")
        eng = nc.sync if kc % 2 else nc.scalar
        eng.dma_start(t[:, :], w_qkv[kc * 128 : (kc + 1) * 128, QKW:F3])
        wv_t.append(t)
    wp_t = []
    for kc in range(KE):
        t = wp_pool.tile([128, E], HF, name=f"wp{kc}", tag="wp")
        nc.scalar.dma_start(t[:, :], w_proj[kc * 128 : (kc + 1) * 128, :])
        wp_t.append(t)
    bias_bc = const_pool.tile([128, E], FP, name="bias_bc", tag="bias_bc")
    nc.scalar.dma_start(bias_bc[:, :], b_proj.unsqueeze(0).broadcast_to([128, E]))

    xT_next = alloc_xT()
    for ti in range(len(TCH)):
        transpose_x_tile(xin_next, xT_next, ti)

    v_next = None
    for b in range(BL):
        xT = xT_next
        if b + 1 < BL:
            xin_next = load_x(b + 1)
            xT_next = alloc_xT()

        def make_pair(hp):
            # q/k tiles for head pair hp: f-chunks hp (q) and 6+hp (k)
            pair = {}
            for nm, fc in (("k", KE + hp), ("q", hp)):
                ps = ps1.tile([128, E], FP, name="psqk", tag="p2", bufs=3)
                for kc in range(KE):
                    for ci, (fs, fw) in enumerate(NCH):
                        nc.tensor.matmul(
                            ps[:, fs : fs + fw],
                            wqk_t[kc][:, fc * 128 : (fc + 1) * 128],
                            xT[kc][:, fs : fs + fw],
                            start=(kc == 0),
                            stop=(kc == KE - 1),
                        )
                t = qk_pool.tile([128, N], HF, name=f"{nm}pair", tag="qk")
                nc.vector.tensor_copy(t[:, :], ps[:, :N])
                pair[nm] = t
            return pair

        # pair 0 before the v matmuls: on batch 0 the q/k weight columns
        # land ~10us before the v columns, so this is the earliest PE work
        pair0 = make_pair(0)

        v_t = emit_v(xT) if b == 0 else v_next

        # attn output in transposed [e, tok] layout, written per head by the
        # normalization chain below
        aoT = [
            aot_pool.tile([128, N], HF, name=f"aoT{kc}", tag="aoT")
            for kc in range(KE)
        ]

        pending = []  # [(attT_tiles, head)] awaiting att@v, 2-deep
        norm_pending = []  # [(ar, rbc, h)] awaiting reciprocal+multiply

        def flush_norm(item):
            # deferred by one head so the GpSimd broadcast has a full head
            # time to land -- an immediate reciprocal would stall the
            # in-order DVE queue (and every score-critical copy behind it)
            # on the cross-engine dependency
            ar, rbc, h = item
            with nc.allow_low_precision(
                reason="fp16 recip of fp16 denominators; 5e-4 rel vs 2e-2 budget"
            ):
                nc.vector.reciprocal(rbc[:, :], rbc[:, :])
            po = (h % 2) * D
            nc.vector.tensor_mul(
                aoT[h // 2][po : po + D, :], ar[D : 2 * D, :N], rbc[D : 2 * D, :]
            )

        def emit_attv(attT_tiles, h):
            # outT[d, i] = sum_j v_ext[j, d] attT[j, i]; psum row 0 = softmax
            # denominator, rows 64:128 = raw output (v_ext layout above)
            pso = ps1.tile([128, E], FP, name="psoT", tag="p2", bufs=3)
            for jc, (js, jw) in enumerate(TCH):
                for ci, (fs, fw) in enumerate(NCH):
                    nc.tensor.matmul(
                        pso[:, fs : fs + fw],
                        v_t[jc][:jw, h, :],
                        attT_tiles[jc][:jw, fs : fs + fw],
                        start=(jc == 0),
                        stop=(jc == len(TCH) - 1),
                    )
            # drain psum with ONE partition-parallel copy. The host-side
            # 1/256 v-scale keeps raw sums and denominators in fp16 range,
            # so the whole normalization chain runs in fp16 (half the bytes
            # on GpSimd, 2x/4x DVE modes on reciprocal and multiply).
            ar = araw_pool.tile([128, N], HF, name="araw", tag="araw")
            nc.vector.tensor_copy(ar[:, :], pso[:, :N])
            # denom varies along the free dim so per-partition scaling is
            # impossible: broadcast the raw denom row across partitions on
            # the idle GpSimd engine (its ucode reads/writes partition 0
            # only, hence the layout gymnastics above), reciprocal
            # lane-parallel on DVE, multiply into aoT. The mul's two SBUF
            # inputs sit at equal aligned base partition 64.
            rbc = rbc_pool.tile([128, N], HF, name="rbc", tag="rbc")
            nc.gpsimd.partition_broadcast(rbc[:, :], ar[0:1, :], channels=128)
            norm_pending.append((ar, rbc, h))
            if len(norm_pending) > 1:
                flush_norm(norm_pending.pop(0))

        for hp in range(H // 2):
            pair = pair0 if hp == 0 else make_pair(hp)

            # att@v of a pending head goes RIGHT AFTER the pair matmuls:
            # its ~1.25us of PE work covers the DVE q/k psum->SBUF copies,
            # so the first score ldweights below never waits on them
            if len(pending) >= 2:
                emit_attv(*pending.pop(0))

            if hp < len(TCH) and b + 1 < BL:
                # transpose one tile of the next batch's x per head-pair,
                # spreading the PE/DVE load (DMAs were issued at batch
                # start); starting at hp=0 lands the last xT copy a full
                # pair before the v matmuls consume the xT tiles
                transpose_x_tile(xin_next, xT_next, hp)

            for sub in range(2):
                h = 2 * hp + sub
                po = sub * D
                q_ap = pair["q"][po : po + D, :]
                k_ap = pair["k"][po : po + D, :]

                attT = [
                    att_pool.tile([128, N], HF, name=f"attT{jc}", tag="attT")
                    for jc in range(len(TCH))
                ]
                for jc, (js, jw) in enumerate(TCH):
                    ps = ps1.tile([128, E], FP, name="pssc", tag="p2", bufs=3)
                    for ci, (fs, fw) in enumerate(NCH):
                        nc.tensor.matmul(
                            ps[:jw, fs : fs + fw],
                            k_ap[:, js : js + jw],
                            q_ap[:, fs : fs + fw],
                            start=True,
                            stop=True,
                        )
                    # one exp per tile (halves the ACT instruction count)
                    nc.scalar.activation(
                        attT[jc][:jw, :],
                        ps[:jw, :N],
                        mybir.ActivationFunctionType.Exp,
                        scale=SCALE,
                    )

                pending.append((attT, h))
                if sub == 0 and len(pending) >= 2:
                    # second pending head between the two subs: pure filler,
                    # the sub-1 scores reuse the already-loaded pair tiles
                    emit_attv(*pending.pop(0))

        for p in pending:
            emit_attv(*p)
        pending = []

        if b + 1 < BL:
            # next batch's v fills PE while the last heads' normalization
            # chains (DVE copies -> GpSimd broadcast -> DVE recip+mul)
            # complete
            v_next = emit_v(xT_next)

        for item in norm_pending:
            flush_norm(item)
        norm_pending = []

        # ---- project, bias, store (aoT already in lhsT layout) ----
        for ti, (ts_, tw) in enumerate(TCH):
            psy0 = ps1.tile([128, E], FP, name="psy0", tag="p2", bufs=3)
            psy = [psy0[:, fs : fs + fw] for (fs, fw) in ECH]
            for kc in range(KE):
                for ci, (fs, fw) in enumerate(ECH):
                    nc.tensor.matmul(
                        psy[ci][:tw, :fw],
                        aoT[kc][:, ts_ : ts_ + tw],
                        wp_t[kc][:, fs : fs + fw],
                        start=(kc == 0),
                        stop=(kc == KE - 1),
                    )
            yt = y_pool.tile([128, E], FP, name="yt", tag="yt")
            # single bias-add drain across both psum banks
            nc.vector.tensor_add(
                yt[:tw, :], psy0[:tw, :E], bias_bc[:tw, :]
            )
            # y stores ride the scalar HWDGE queue: the SP queue carries the
            # next batch's x loads and must not queue behind 5 y stores
            nc.scalar.dma_start(y[b, ts_ : ts_ + tw, :], yt[:tw, :])


_NC_CACHE = None


def _dedupe_ldweights(nc):
    """Drop InstLdweights whose stationary AP is identical to the previous
    load on the PE stream (the NCH/ECH matmul pairs in this kernel reload
    the same weights back-to-back). The PE array retains loaded weights
    until the next LDWEIGHTS, so the paired matmul executes identically;
    ~1000 redundant weight loads (>=50ns each on HW) disappear. Runs
    pre-compile, where the dedupable loads carry no sync waits/updates
    (the wait-redistribution pass runs inside nc.compile() afterwards)."""
    removed = 0
    for blk in nc.m.functions[0].blocks:
        last = None
        to_remove = []
        for ins in blk.instructions:
            if type(ins).__name__ != "InstLdweights":
                continue
            si = ins.sync_info
            clean = si is None or (len(si.on_wait) == 0 and len(si.on_update) == 0)
            key = (
                str(ins.ins[0]),
                str(ins.tile_size),
                str(ins.tile_position),
                str(ins.perf_mode),
                str(ins.is_transpose),
            )
            if key == last and clean:
                to_remove.append(ins)
                removed += 1
            else:
                last = key
        for ins in to_remove:
            blk.instructions.remove(ins)
    return removed


def build_program():
    global _NC_CACHE
    if _NC_CACHE is not None:
        return _NC_CACHE
    from contextlib import ExitStack

    nc = bacc.Bacc(
        trn_type="TRN2", target_bir_lowering=False, debug=False, num_devices=NCORES
    )
    x = nc.dram_tensor("x", [BL, N, E], HF, kind="ExternalInput").ap()
    w_qkv = nc.dram_tensor("w_qkv", [E, F3], HF, kind="ExternalInput").ap()
    w_proj = nc.dram_tensor("w_proj", [E, E], HF, kind="ExternalInput").ap()
    b_proj = nc.dram_tensor("b_proj", [E], FP, kind="ExternalInput").ap()
    y = nc.dram_tensor("y", [BL, N, E], FP, kind="ExternalOutput").ap()

    with tile.TileContext(nc) as tc:
        with ExitStack() as ctx:
            _emit(tc, x, w_qkv, w_proj, b_proj, y, ctx)
    n_removed = _dedupe_ldweights(nc)
    assert n_removed > 900, f"ldweights dedup removed only {n_removed}"
    # splits excess sync waits (1-per-instruction HW limit) via ldweights /
    # event-semaphore carriers, among other lowering passes
    nc.compile()

    _NC_CACHE = nc
    return nc


def kernel(x, w_qkv, w_proj, b_proj, _trace=False, _tmpdir=None):
    nc = build_program()
    # fp16 conversion happens host-side (the kernel consumed fp16 operands
    # anyway); the v columns of w_qkv are pre-scaled by 1/256 so the fp32
    # attention sums fit fp16 on the way out of PSUM -- the matching 1/256
    # on the in-kernel ones column cancels it exactly in the normalize.
    x16 = np.ascontiguousarray(np.asarray(x, dtype=np.float32)).astype(np.float16)
    w_qkv16 = np.asarray(w_qkv, dtype=np.float32).copy()
    w_qkv16[:, 2 * E :] *= 1.0 / 256.0
    w_qkv16 = np.ascontiguousarray(w_qkv16).astype(np.float16)
    w_proj16 = np.ascontiguousarray(
        np.asarray(w_proj, dtype=np.float32)
    ).astype(np.float16)
    in_maps = [
        {
            "x": np.ascontiguousarray(x16[i * BL : (i + 1) * BL]),
            "w_qkv": w_qkv16,
            "w_proj": w_proj16,
            "b_proj": np.ascontiguousarray(b_proj, dtype=np.float32),
        }
        for i in range(NCORES)
    ]
    res = run_bass_kernel_spmd(
        nc, in_maps, core_ids=list(range(NCORES)), trace=_trace, tmpdir=_tmpdir
    )
    out = np.concatenate([r["y"] for r in res.results], axis=0)
    if _trace:
        kernel.last_results = res
    return out

